# revision 13
# baseline (speedup 1.0000x reference)
"""DeepClusterLoss on 8 Trainium2 NeuronCores (Bass/Tile).

reference:
    recon_loss   = sum((recon_x - x)**2)
    cluster_loss = sum((x - centers[assign])**2)
    total        = recon_loss + cluster_loss          (ALPHA = BETA = 1)

Device strategy (data-parallel over N, per the sharding hint):
  - Inputs are streamed in bf16 (host-side cast, exact-to-tolerance: all
    outputs are ~1e8-magnitude sums of ~1e0 terms; the bf16 rounding noise
    averages to ~1e-6 relative).  This halves HBM traffic and unlocks the
    fast PE/DVE paths (1 cycle/row matmuls, single-pass LDWEIGHTS, 2x DVE).
  - Each sample is stored as 65 bf16s: [x_i (64) | flag], flag = 1.0 for
    real samples, 0.0 for padding.  recon_x rows carry the same flag, so
    (r - x) has an exact 0 in the flag column.
  - recon part: DVE computes d = r - x (bf16), ACT computes Square(d) with
    accum_out -> fp32 per-partition partials.  ACT Square(x) likewise (the
    flag column adds +1 per real sample; the host subtracts N afterwards).
  - cluster part avoids the gather:
        cluster = sum|x|^2 - 2*sum_k <S_k, C_k> + sum_k n_k*|C_k|^2
    S_k (segment sums) and n_k (counts) come from ONE matmul per
    128-sample slot: a one-hot [128, K] bf16 (tensor_scalar is_equal
    against an iota row; built on DVE and GpSimd in parallel) contracted
    with the augmented x-slot [128, 65] -> PSUM [K, 65] fp32, where column
    64 (the flag) accumulates exactly n_k.
  - Host combines the tiny per-core fp32 outputs in float64.

Padding uses assignment class K (=100): its one-hot row is all zeros, so
padded samples vanish from S and the counts.
"""

import sys
from contextlib import ExitStack

import numpy as np

for _p in ("/opt/trn_rl_repo", "/opt/pypackages"):
    if _p not in sys.path:
        sys.path.append(_p)

import ml_dtypes
import concourse.tile as tile
from concourse import bacc, mybir
from concourse.bass_utils import run_bass_kernel_spmd

N, D, K = 1_000_000, 64, 100
ALPHA, BETA = 1.0, 1.0
N_CORES = 8
N_PER_CORE = N // N_CORES  # 125000
P = 128                    # SBUF partitions
DA = D + 1                 # augmented sample width (x | flag)
SLOTS = 32                 # sample-slots per partition per tile
FREE = DA * SLOTS          # 2080 bf16 per partition per half-tile
SPT = P * SLOTS            # samples per tile = 4096
NTILES = -(-N_PER_CORE // SPT)  # 31
PADDED = NTILES * SPT      # 126976
PAD_CLASS = float(K)       # out-of-range class: one-hot row is all zeros
GP_FRAC = 3                # j % GP_FRAC == 0 -> one-hot built on GpSimd

_bf16 = mybir.dt.bfloat16
_f32 = mybir.dt.float32
BF16 = ml_dtypes.bfloat16


def build_nc(ntiles: int = NTILES):
    """Build + compile the per-core Bass program (same program on all cores)."""
    nc = bacc.Bacc()
    # x-aug and r-aug interleaved per tile: xr[t, p, 0:FREE] = x-aug,
    # xr[t, p, FREE:2*FREE] = r-aug  (one DMA per tile)
    xr_d = nc.dram_tensor("xr", [ntiles, P, 2 * FREE], _bf16, kind="ExternalInput")
    # bf16 iota rows for the one-hot compares; fp32 slot-layout assignments
    # (the tensor_scalar Ptr operand must be fp32)
    iota_d = nc.dram_tensor("iota", [P, K], _bf16, kind="ExternalInput")
    a_d = nc.dram_tensor("a", [P, ntiles * SLOTS], _f32, kind="ExternalInput")
    s_out = nc.dram_tensor("s_out", [K, DA], _f32, kind="ExternalOutput")
    part_out = nc.dram_tensor("partials", [P, 2 * ntiles], _f32, kind="ExternalOutput")

    with ExitStack() as ctx:
        tc = ctx.enter_context(tile.TileContext(nc))
        const_pool = ctx.enter_context(tc.tile_pool(name="const", bufs=1))
        xin = ctx.enter_context(tc.tile_pool(name="xin", bufs=6))
        scratch = ctx.enter_context(tc.tile_pool(name="scratch", bufs=2))
        ohp = ctx.enter_context(tc.tile_pool(name="ohp", bufs=3))
        psum = ctx.enter_context(tc.tile_pool(name="psum", bufs=1, space="PSUM"))

        iota_tile = const_pool.tile([P, K], _bf16)
        nc.sync.dma_start(iota_tile[:], iota_d[:, :])
        iota_sb = iota_tile[:]
        a_sb = const_pool.tile([P, ntiles * SLOTS], _f32)
        nc.sync.dma_start(a_sb[:], a_d[:, :])
        partials_sb = const_pool.tile([P, 2 * ntiles], _f32)

        s_psum = psum.tile([K, DA], _f32)

        for t in range(ntiles):
            xr_t = xin.tile([P, 2 * FREE], _bf16)
            nc.sync.dma_start(xr_t[:], xr_d[t, :, :])
            x_t = xr_t[:, 0:FREE]
            r_t = xr_t[:, FREE : 2 * FREE]

            d_t = scratch.tile([P, FREE], _bf16, tag="d")
            nc.vector.tensor_sub(d_t[:], r_t, x_t)
            sq_t = scratch.tile([P, FREE], _bf16, tag="sq")
            nc.scalar.activation(
                sq_t[:], d_t[:], mybir.ActivationFunctionType.Square,
                accum_out=partials_sb[:, t : t + 1],
            )
            sq2_t = scratch.tile([P, FREE], _bf16, tag="sq")
            nc.scalar.activation(
                sq2_t[:], x_t, mybir.ActivationFunctionType.Square,
                accum_out=partials_sb[:, ntiles + t : ntiles + t + 1],
            )

            oh_big = ohp.tile([P, SLOTS * K], _bf16)
            for j in range(SLOTS):
                oh = oh_big[:, j * K : (j + 1) * K]
                col = t * SLOTS + j
                eng = nc.gpsimd if j % GP_FRAC == 0 else nc.vector
                eng.tensor_scalar(
                    out=oh, in0=iota_sb,
                    scalar1=a_sb[:, col : col + 1], scalar2=None,
                    op0=mybir.AluOpType.is_equal,
                )
                nc.tensor.matmul(
                    s_psum[:],
                    oh,
                    x_t[:, j * DA : (j + 1) * DA],
                    start=(t == 0 and j == 0),
                    stop=(t == ntiles - 1 and j == SLOTS - 1),
                )

        s_sb = const_pool.tile([K, DA], _f32)
        nc.vector.tensor_copy(s_sb[:], s_psum[:])
        nc.sync.dma_start(s_out[:, :], s_sb[:])
        nc.sync.dma_start(part_out[:, :], partials_sb[:])

    nc.compile()
    return nc


def host_prepare(recon_x, x, cluster_assignments, ntiles: int = NTILES,
                 n_cores: int = N_CORES):
    """Shard + pad + cast + lay out the inputs for each core."""
    n_per_core = x.shape[0] // n_cores
    padded = ntiles * SPT
    x_np = np.asarray(x, dtype=np.float32).reshape(n_cores, n_per_core, D)
    r_np = np.asarray(recon_x, dtype=np.float32).reshape(n_cores, n_per_core, D)
    a_np = np.asarray(cluster_assignments).reshape(n_cores, n_per_core)

    xr = np.zeros((n_cores, ntiles, P, 2 * FREE), BF16)
    xa = np.zeros((n_cores, padded, DA), BF16)
    xa[:, :n_per_core, :D] = x_np.astype(BF16)
    xa[:, :n_per_core, D] = 1.0
    xr[:, :, :, 0:FREE] = xa.reshape(n_cores, ntiles, P, FREE)
    xa[:, :n_per_core, :D] = r_np.astype(BF16)   # reuse buffer for r-aug
    xr[:, :, :, FREE:] = xa.reshape(n_cores, ntiles, P, FREE)

    As = np.full((n_cores, padded), PAD_CLASS, np.float32)
    As[:, :n_per_core] = a_np.astype(np.float32)

    iota_np = np.ascontiguousarray(np.broadcast_to(np.arange(K, dtype=BF16), (P, K)))

    in_maps = []
    for c in range(n_cores):
        a_arr = np.ascontiguousarray(
            As[c].reshape(ntiles, P, SLOTS).transpose(1, 0, 2).reshape(P, -1)
        )
        in_maps.append({"xr": xr[c], "iota": iota_np, "a": a_arr})
    return in_maps


def host_combine(results, cluster_centers, ntiles: int = NTILES,
                 n_real: int = N):
    """Reduce per-core outputs into (total, recon, cluster) in float64."""
    S = np.zeros((K, DA), np.float64)
    recon = 0.0
    xsq = 0.0
    for rd in results:
        S += rd["s_out"].astype(np.float64)
        pr = rd["partials"].astype(np.float64)
        recon += pr[:, :ntiles].sum()
        xsq += pr[:, ntiles:].sum()
    xsq -= n_real  # flag column contributes 1 per real sample
    cnt = S[:, D]
    C = np.asarray(cluster_centers, dtype=np.float64)
    cross = float((S[:, :D] * C).sum())
    w = (C * C).sum(axis=1)
    cluster = xsq - 2.0 * cross + float((cnt * w).sum())
    total = ALPHA * recon + BETA * cluster
    return (np.float32(total), np.float32(recon), np.float32(cluster))


_nc = None


def _get_nc():
    global _nc
    if _nc is None:
        _nc = build_nc()
    return _nc


def kernel(recon_x, x, cluster_assignments, cluster_centers):
    nc = _get_nc()
    in_maps = host_prepare(recon_x, x, cluster_assignments)
    res = run_bass_kernel_spmd(nc, in_maps, list(range(N_CORES)))
    return host_combine(res.results, cluster_centers)


# revision 17
# speedup vs baseline: 1.1581x; 1.1581x over previous
"""DeepClusterLoss on 8 Trainium2 NeuronCores (Bass/Tile).

reference:
    recon_loss   = sum((recon_x - x)**2)
    cluster_loss = sum((x - centers[assign])**2)
    total        = recon_loss + cluster_loss          (ALPHA = BETA = 1)

Device strategy (data-parallel over N, per the sharding hint):
  - Inputs are streamed in bf16 (host-side cast, exact-to-tolerance: all
    outputs are ~1e8-magnitude sums of ~1e0 terms; the bf16 rounding noise
    averages to ~1e-6 relative).  This halves HBM traffic and unlocks the
    fast PE/DVE paths (1 cycle/row matmuls, single-pass LDWEIGHTS, 2x DVE).
  - Each sample is stored as 65 bf16s: [x_i (64) | flag], flag = 1.0 for
    real samples, 0.0 for padding.  recon_x rows carry the same flag, so
    (r - x) has an exact 0 in the flag column.
  - recon part: DVE computes d = r - x (bf16), ACT computes Square(d) with
    accum_out -> fp32 per-partition partials.  ACT Square(x) likewise (the
    flag column adds +1 per real sample; the host subtracts N afterwards).
  - cluster part avoids the gather:
        cluster = sum|x|^2 - 2*sum_k <S_k, C_k> + sum_k n_k*|C_k|^2
    S_k (segment sums) and n_k (counts) come from ONE matmul per
    128-sample slot: a one-hot [128, K] bf16 (tensor_scalar is_equal
    against an iota row; built on DVE and GpSimd in parallel) contracted
    with the augmented x-slot [128, 65] -> PSUM [K, 65] fp32, where column
    64 (the flag) accumulates exactly n_k.
  - Host combines the tiny per-core fp32 outputs in float64.

Padding uses assignment class K (=100): its one-hot row is all zeros, so
padded samples vanish from S and the counts.
"""

import sys
from contextlib import ExitStack

import numpy as np

for _p in ("/opt/trn_rl_repo", "/opt/pypackages"):
    if _p not in sys.path:
        sys.path.append(_p)

import ml_dtypes
import concourse.tile as tile
from concourse import bacc, mybir
from concourse.bass_utils import run_bass_kernel_spmd

N, D, K = 1_000_000, 64, 100
ALPHA, BETA = 1.0, 1.0
N_CORES = 8
N_PER_CORE = N // N_CORES  # 125000
P = 128                    # SBUF partitions
DA = D + 1                 # augmented sample width (x | flag)
SLOTS = 32                 # sample-slots per partition per tile
FREE = DA * SLOTS          # 2080 bf16 per partition per half-tile
SPT = P * SLOTS            # samples per tile = 4096
NTILES = -(-N_PER_CORE // SPT)  # 31
PADDED = NTILES * SPT      # 126976
PAD_CLASS = float(K)       # out-of-range class: one-hot row is all zeros
GP_FRAC = 3                # j % GP_FRAC == 0 -> one-hot built on GpSimd

_bf16 = mybir.dt.bfloat16
_f32 = mybir.dt.float32
BF16 = ml_dtypes.bfloat16


def build_nc(ntiles: int = NTILES):
    """Build + compile the per-core Bass program (same program on all cores)."""
    nc = bacc.Bacc()
    # x-aug and r-aug interleaved per tile: xr[t, p, 0:FREE] = x-aug,
    # xr[t, p, FREE:2*FREE] = r-aug  (one DMA per tile)
    xr_d = nc.dram_tensor("xr", [ntiles, P, 2 * FREE], _bf16, kind="ExternalInput")
    # fp32 iota rows for the one-hot compares (the fp32 tensor_scalar path is
    # the fast DVE path; bf16 tensor_scalar falls into a slow ucode path);
    # fp32 slot-layout assignments (the tensor_scalar Ptr operand must be fp32)
    iota_d = nc.dram_tensor("iota", [P, K], _f32, kind="ExternalInput")
    a_d = nc.dram_tensor("a", [P, ntiles * SLOTS], _f32, kind="ExternalInput")
    s_out = nc.dram_tensor("s_out", [K, DA], _f32, kind="ExternalOutput")
    part_out = nc.dram_tensor("partials", [P, 2 * ntiles], _f32, kind="ExternalOutput")

    with ExitStack() as ctx:
        tc = ctx.enter_context(tile.TileContext(nc))
        const_pool = ctx.enter_context(tc.tile_pool(name="const", bufs=1))
        xin = ctx.enter_context(tc.tile_pool(name="xin", bufs=6))
        scratch = ctx.enter_context(tc.tile_pool(name="scratch", bufs=2))
        ohp = ctx.enter_context(tc.tile_pool(name="ohp", bufs=3))
        psum = ctx.enter_context(tc.tile_pool(name="psum", bufs=1, space="PSUM"))

        iota_tile = const_pool.tile([P, K], _f32)
        nc.sync.dma_start(iota_tile[:], iota_d[:, :])
        iota_sb = iota_tile[:]
        a_sb = const_pool.tile([P, ntiles * SLOTS], _f32)
        nc.sync.dma_start(a_sb[:], a_d[:, :])
        partials_sb = const_pool.tile([P, 2 * ntiles], _f32)

        s_psum = psum.tile([K, DA], _f32)

        for t in range(ntiles):
            xr_t = xin.tile([P, 2 * FREE], _bf16)
            nc.sync.dma_start(xr_t[:], xr_d[t, :, :])
            x_t = xr_t[:, 0:FREE]
            r_t = xr_t[:, FREE : 2 * FREE]

            d_t = scratch.tile([P, FREE], _bf16, tag="d")
            nc.vector.tensor_sub(d_t[:], r_t, x_t)
            sq_t = scratch.tile([P, FREE], _bf16, tag="sq")
            nc.scalar.activation(
                sq_t[:], d_t[:], mybir.ActivationFunctionType.Square,
                accum_out=partials_sb[:, t : t + 1],
            )
            sq2_t = scratch.tile([P, FREE], _bf16, tag="sq")
            nc.scalar.activation(
                sq2_t[:], x_t, mybir.ActivationFunctionType.Square,
                accum_out=partials_sb[:, ntiles + t : ntiles + t + 1],
            )

            oh_big = ohp.tile([P, SLOTS * K], _f32, tag="ohf")
            for j in range(SLOTS):
                col = t * SLOTS + j
                nc.vector.tensor_scalar(
                    out=oh_big[:, j * K : (j + 1) * K], in0=iota_sb,
                    scalar1=a_sb[:, col : col + 1], scalar2=None,
                    op0=mybir.AluOpType.is_equal,
                )
            # bulk fp32 -> bf16 cast on the otherwise-idle GpSimd engine
            oh_bf = ohp.tile([P, SLOTS * K], _bf16, tag="ohb")
            nc.gpsimd.tensor_copy(oh_bf[:], oh_big[:])
            for j in range(SLOTS):
                nc.tensor.matmul(
                    s_psum[:],
                    oh_bf[:, j * K : (j + 1) * K],
                    x_t[:, j * DA : (j + 1) * DA],
                    start=(t == 0 and j == 0),
                    stop=(t == ntiles - 1 and j == SLOTS - 1),
                )

        s_sb = const_pool.tile([K, DA], _f32)
        nc.vector.tensor_copy(s_sb[:], s_psum[:])
        nc.sync.dma_start(s_out[:, :], s_sb[:])
        nc.sync.dma_start(part_out[:, :], partials_sb[:])

    nc.compile()
    return nc


def host_prepare(recon_x, x, cluster_assignments, ntiles: int = NTILES,
                 n_cores: int = N_CORES):
    """Shard + pad + cast + lay out the inputs for each core."""
    n_per_core = x.shape[0] // n_cores
    padded = ntiles * SPT
    x_np = np.asarray(x, dtype=np.float32).reshape(n_cores, n_per_core, D)
    r_np = np.asarray(recon_x, dtype=np.float32).reshape(n_cores, n_per_core, D)
    a_np = np.asarray(cluster_assignments).reshape(n_cores, n_per_core)

    xr = np.zeros((n_cores, ntiles, P, 2 * FREE), BF16)
    xa = np.zeros((n_cores, padded, DA), BF16)
    xa[:, :n_per_core, :D] = x_np.astype(BF16)
    xa[:, :n_per_core, D] = 1.0
    xr[:, :, :, 0:FREE] = xa.reshape(n_cores, ntiles, P, FREE)
    xa[:, :n_per_core, :D] = r_np.astype(BF16)   # reuse buffer for r-aug
    xr[:, :, :, FREE:] = xa.reshape(n_cores, ntiles, P, FREE)

    As = np.full((n_cores, padded), PAD_CLASS, np.float32)
    As[:, :n_per_core] = a_np.astype(np.float32)

    iota_np = np.ascontiguousarray(
        np.broadcast_to(np.arange(K, dtype=np.float32), (P, K))
    )

    in_maps = []
    for c in range(n_cores):
        a_arr = np.ascontiguousarray(
            As[c].reshape(ntiles, P, SLOTS).transpose(1, 0, 2).reshape(P, -1)
        )
        in_maps.append({"xr": xr[c], "iota": iota_np, "a": a_arr})
    return in_maps


def host_combine(results, cluster_centers, ntiles: int = NTILES,
                 n_real: int = N):
    """Reduce per-core outputs into (total, recon, cluster) in float64."""
    S = np.zeros((K, DA), np.float64)
    recon = 0.0
    xsq = 0.0
    for rd in results:
        S += rd["s_out"].astype(np.float64)
        pr = rd["partials"].astype(np.float64)
        recon += pr[:, :ntiles].sum()
        xsq += pr[:, ntiles:].sum()
    xsq -= n_real  # flag column contributes 1 per real sample
    cnt = S[:, D]
    C = np.asarray(cluster_centers, dtype=np.float64)
    cross = float((S[:, :D] * C).sum())
    w = (C * C).sum(axis=1)
    cluster = xsq - 2.0 * cross + float((cnt * w).sum())
    total = ALPHA * recon + BETA * cluster
    return (np.float32(total), np.float32(recon), np.float32(cluster))


_nc = None


def _get_nc():
    global _nc
    if _nc is None:
        _nc = build_nc()
    return _nc


def kernel(recon_x, x, cluster_assignments, cluster_centers):
    nc = _get_nc()
    in_maps = host_prepare(recon_x, x, cluster_assignments)
    res = run_bass_kernel_spmd(nc, in_maps, list(range(N_CORES)))
    return host_combine(res.results, cluster_centers)


# revision 18
# speedup vs baseline: 2.6359x; 2.2760x over previous
"""DeepClusterLoss on 8 Trainium2 NeuronCores (Bass/Tile).

reference:
    recon_loss   = sum((recon_x - x)**2)
    cluster_loss = sum((x - centers[assign])**2)
    total        = recon_loss + cluster_loss          (ALPHA = BETA = 1)

Device strategy (data-parallel over N, per the sharding hint):
  - Inputs are streamed in bf16 (host-side cast, exact-to-tolerance: all
    outputs are ~1e8-magnitude sums of ~1e0 terms; the bf16 rounding noise
    averages to ~1e-6 relative).  This halves HBM traffic and unlocks the
    fast PE/DVE paths (1 cycle/row matmuls, single-pass LDWEIGHTS, 2x DVE).
  - Each sample is stored as 65 bf16s: [x_i (64) | flag], flag = 1.0 for
    real samples, 0.0 for padding.  recon_x rows carry the same flag, so
    (r - x) has an exact 0 in the flag column.
  - recon part: DVE computes d = r - x (bf16), ACT computes Square(d) with
    accum_out -> fp32 per-partition partials.  ACT Square(x) likewise (the
    flag column adds +1 per real sample; the host subtracts N afterwards).
  - cluster part avoids the gather:
        cluster = sum|x|^2 - 2*sum_k <S_k, C_k> + sum_k n_k*|C_k|^2
    S_k (segment sums) and n_k (counts) come from ONE matmul per
    128-sample slot: a one-hot [128, K] bf16 (tensor_scalar is_equal
    against an iota row; built on DVE and GpSimd in parallel) contracted
    with the augmented x-slot [128, 65] -> PSUM [K, 65] fp32, where column
    64 (the flag) accumulates exactly n_k.
  - Host combines the tiny per-core fp32 outputs in float64.

Padding uses assignment class K (=100): its one-hot row is all zeros, so
padded samples vanish from S and the counts.
"""

import sys
from contextlib import ExitStack

import numpy as np

for _p in ("/opt/trn_rl_repo", "/opt/pypackages"):
    if _p not in sys.path:
        sys.path.append(_p)

import ml_dtypes
import concourse.tile as tile
from concourse import bacc, mybir
from concourse.bass_utils import run_bass_kernel_spmd

N, D, K = 1_000_000, 64, 100
ALPHA, BETA = 1.0, 1.0
N_CORES = 8
N_PER_CORE = N // N_CORES  # 125000
P = 128                    # SBUF partitions
DA = D + 1                 # augmented sample width (x | flag)
SLOTS = 32                 # sample-slots per partition per tile
FREE = DA * SLOTS          # 2080 bf16 per partition per half-tile
SPT = P * SLOTS            # samples per tile = 4096
NTILES = -(-N_PER_CORE // SPT)  # 31
PADDED = NTILES * SPT      # 126976
PAD_CLASS = float(K)       # out-of-range class: one-hot row is all zeros
GP_FRAC = 3                # j % GP_FRAC == 0 -> one-hot built on GpSimd

_bf16 = mybir.dt.bfloat16
_f32 = mybir.dt.float32
BF16 = ml_dtypes.bfloat16


def build_nc(ntiles: int = NTILES):
    """Build + compile the per-core Bass program (same program on all cores)."""
    nc = bacc.Bacc()
    # x-aug and r-aug interleaved per tile: xr[t, p, 0:FREE] = x-aug,
    # xr[t, p, FREE:2*FREE] = r-aug  (one DMA per tile)
    xr_d = nc.dram_tensor("xr", [ntiles, P, 2 * FREE], _bf16, kind="ExternalInput")
    # fp32 iota rows for the one-hot compares (the fp32 tensor_scalar path is
    # the fast DVE path; bf16 tensor_scalar falls into a slow ucode path);
    # fp32 slot-layout assignments (the tensor_scalar Ptr operand must be fp32)
    iota_d = nc.dram_tensor("iota", [P, K], _f32, kind="ExternalInput")
    a_d = nc.dram_tensor("a", [P, ntiles * SLOTS], _f32, kind="ExternalInput")
    s_out = nc.dram_tensor("s_out", [K, DA], _f32, kind="ExternalOutput")
    part_out = nc.dram_tensor("partials", [P, 2 * ntiles], _f32, kind="ExternalOutput")

    with ExitStack() as ctx:
        tc = ctx.enter_context(tile.TileContext(nc))
        const_pool = ctx.enter_context(tc.tile_pool(name="const", bufs=1))
        xin = ctx.enter_context(tc.tile_pool(name="xin", bufs=6))
        scratch = ctx.enter_context(tc.tile_pool(name="scratch", bufs=2))
        ohp = ctx.enter_context(tc.tile_pool(name="ohp", bufs=3))
        psum = ctx.enter_context(tc.tile_pool(name="psum", bufs=1, space="PSUM"))

        iota_tile = const_pool.tile([P, K], _f32)
        nc.sync.dma_start(iota_tile[:], iota_d[:, :])
        iota_sb = iota_tile[:]
        a_sb = const_pool.tile([P, ntiles * SLOTS], _f32)
        nc.sync.dma_start(a_sb[:], a_d[:, :])
        partials_sb = const_pool.tile([P, 2 * ntiles], _f32)

        s_psum = psum.tile([K, DA], _f32)

        for t in range(ntiles):
            xr_t = xin.tile([P, 2 * FREE], _bf16)
            nc.sync.dma_start(xr_t[:], xr_d[t, :, :])
            x_t = xr_t[:, 0:FREE]
            r_t = xr_t[:, FREE : 2 * FREE]

            d_t = scratch.tile([P, FREE], _bf16, tag="d")
            nc.vector.tensor_sub(d_t[:], r_t, x_t)
            sq_t = scratch.tile([P, FREE], _bf16, tag="sq")
            nc.scalar.activation(
                sq_t[:], d_t[:], mybir.ActivationFunctionType.Square,
                accum_out=partials_sb[:, t : t + 1],
            )
            sq2_t = scratch.tile([P, FREE], _bf16, tag="sq")
            nc.scalar.activation(
                sq2_t[:], x_t, mybir.ActivationFunctionType.Square,
                accum_out=partials_sb[:, ntiles + t : ntiles + t + 1],
            )

            oh_big = ohp.tile([P, SLOTS * K], _f32, tag="ohf")
            for j in range(SLOTS):
                col = t * SLOTS + j
                nc.vector.tensor_scalar(
                    out=oh_big[:, j * K : (j + 1) * K], in0=iota_sb,
                    scalar1=a_sb[:, col : col + 1], scalar2=None,
                    op0=mybir.AluOpType.is_equal,
                )
            # bulk fp32 -> bf16 cast on ACT (dedicated SBUF ports: no
            # contention with the DVE one-hot stream, unlike GpSimd whose
            # port is shared with DVE)
            oh_bf = ohp.tile([P, SLOTS * K], _bf16, tag="ohb")
            nc.scalar.copy(oh_bf[:], oh_big[:])
            for j in range(SLOTS):
                nc.tensor.matmul(
                    s_psum[:],
                    oh_bf[:, j * K : (j + 1) * K],
                    x_t[:, j * DA : (j + 1) * DA],
                    start=(t == 0 and j == 0),
                    stop=(t == ntiles - 1 and j == SLOTS - 1),
                )

        s_sb = const_pool.tile([K, DA], _f32)
        nc.vector.tensor_copy(s_sb[:], s_psum[:])
        nc.sync.dma_start(s_out[:, :], s_sb[:])
        nc.sync.dma_start(part_out[:, :], partials_sb[:])

    nc.compile()
    return nc


def host_prepare(recon_x, x, cluster_assignments, ntiles: int = NTILES,
                 n_cores: int = N_CORES):
    """Shard + pad + cast + lay out the inputs for each core."""
    n_per_core = x.shape[0] // n_cores
    padded = ntiles * SPT
    x_np = np.asarray(x, dtype=np.float32).reshape(n_cores, n_per_core, D)
    r_np = np.asarray(recon_x, dtype=np.float32).reshape(n_cores, n_per_core, D)
    a_np = np.asarray(cluster_assignments).reshape(n_cores, n_per_core)

    xr = np.zeros((n_cores, ntiles, P, 2 * FREE), BF16)
    xa = np.zeros((n_cores, padded, DA), BF16)
    xa[:, :n_per_core, :D] = x_np.astype(BF16)
    xa[:, :n_per_core, D] = 1.0
    xr[:, :, :, 0:FREE] = xa.reshape(n_cores, ntiles, P, FREE)
    xa[:, :n_per_core, :D] = r_np.astype(BF16)   # reuse buffer for r-aug
    xr[:, :, :, FREE:] = xa.reshape(n_cores, ntiles, P, FREE)

    As = np.full((n_cores, padded), PAD_CLASS, np.float32)
    As[:, :n_per_core] = a_np.astype(np.float32)

    iota_np = np.ascontiguousarray(
        np.broadcast_to(np.arange(K, dtype=np.float32), (P, K))
    )

    in_maps = []
    for c in range(n_cores):
        a_arr = np.ascontiguousarray(
            As[c].reshape(ntiles, P, SLOTS).transpose(1, 0, 2).reshape(P, -1)
        )
        in_maps.append({"xr": xr[c], "iota": iota_np, "a": a_arr})
    return in_maps


def host_combine(results, cluster_centers, ntiles: int = NTILES,
                 n_real: int = N):
    """Reduce per-core outputs into (total, recon, cluster) in float64."""
    S = np.zeros((K, DA), np.float64)
    recon = 0.0
    xsq = 0.0
    for rd in results:
        S += rd["s_out"].astype(np.float64)
        pr = rd["partials"].astype(np.float64)
        recon += pr[:, :ntiles].sum()
        xsq += pr[:, ntiles:].sum()
    xsq -= n_real  # flag column contributes 1 per real sample
    cnt = S[:, D]
    C = np.asarray(cluster_centers, dtype=np.float64)
    cross = float((S[:, :D] * C).sum())
    w = (C * C).sum(axis=1)
    cluster = xsq - 2.0 * cross + float((cnt * w).sum())
    total = ALPHA * recon + BETA * cluster
    return (np.float32(total), np.float32(recon), np.float32(cluster))


_nc = None


def _get_nc():
    global _nc
    if _nc is None:
        _nc = build_nc()
    return _nc


def kernel(recon_x, x, cluster_assignments, cluster_centers):
    nc = _get_nc()
    in_maps = host_prepare(recon_x, x, cluster_assignments)
    res = run_bass_kernel_spmd(nc, in_maps, list(range(N_CORES)))
    return host_combine(res.results, cluster_centers)


# revision 22
# speedup vs baseline: 3.3616x; 1.2753x over previous
"""DeepClusterLoss on 8 Trainium2 NeuronCores (Bass/Tile).

reference:
    recon_loss   = sum((recon_x - x)**2)
    cluster_loss = sum((x - centers[assign])**2)
    total        = recon_loss + cluster_loss          (ALPHA = BETA = 1)

Device strategy (data-parallel over N, per the sharding hint):
  - Inputs are streamed in bf16 (host-side cast, exact-to-tolerance: all
    outputs are ~1e8-magnitude sums of ~1e0 terms; the bf16 rounding noise
    averages to ~1e-6 relative).  This halves HBM traffic and unlocks the
    fast PE/DVE paths (1 cycle/row matmuls, single-pass LDWEIGHTS, 2x DVE).
  - Each sample is stored as 65 bf16s: [x_i (64) | flag], flag = 1.0 for
    real samples, 0.0 for padding.  recon_x rows carry the same flag, so
    (r - x) has an exact 0 in the flag column.
  - recon part: DVE computes d = r - x (bf16), ACT computes Square(d) with
    accum_out -> fp32 per-partition partials.  ACT Square(x) likewise (the
    flag column adds +1 per real sample; the host subtracts N afterwards).
  - cluster part avoids the gather:
        cluster = sum|x|^2 - 2*sum_k <S_k, C_k> + sum_k n_k*|C_k|^2
    S_k (segment sums) and n_k (counts) come from ONE matmul per
    128-sample slot: a one-hot [128, K] bf16 (tensor_scalar is_equal
    against an iota row; built on DVE and GpSimd in parallel) contracted
    with the augmented x-slot [128, 65] -> PSUM [K, 65] fp32, where column
    64 (the flag) accumulates exactly n_k.
  - Host combines the tiny per-core fp32 outputs in float64.

Padding uses assignment class K (=100): its one-hot row is all zeros, so
padded samples vanish from S and the counts.
"""

import sys
from contextlib import ExitStack

import numpy as np

for _p in ("/opt/trn_rl_repo", "/opt/pypackages"):
    if _p not in sys.path:
        sys.path.append(_p)

import ml_dtypes
import concourse.tile as tile
from concourse import bacc, mybir
from concourse.bass_utils import run_bass_kernel_spmd

N, D, K = 1_000_000, 64, 100
ALPHA, BETA = 1.0, 1.0
N_CORES = 8
N_PER_CORE = N // N_CORES  # 125000
P = 128                    # SBUF partitions
DA = D + 1                 # augmented sample width (x | flag)
SLOTS = 32                 # sample-slots per partition per tile
FREE = DA * SLOTS          # 2080 bf16 per partition per half-tile
SPT = P * SLOTS            # samples per tile = 4096
NTILES = -(-N_PER_CORE // SPT)  # 31
PADDED = NTILES * SPT      # 126976
PAD_CLASS = float(K)       # out-of-range class: one-hot row is all zeros
GP_FRAC = 3                # j % GP_FRAC == 0 -> one-hot built on GpSimd

_bf16 = mybir.dt.bfloat16
_f32 = mybir.dt.float32
BF16 = ml_dtypes.bfloat16


def build_nc(ntiles: int = NTILES):
    """Build + compile the per-core Bass program (same program on all cores)."""
    nc = bacc.Bacc()
    # x-aug and r-aug interleaved per tile: xr[t, p, 0:FREE] = x-aug,
    # xr[t, p, FREE:2*FREE] = r-aug  (one DMA per tile)
    xr_d = nc.dram_tensor("xr", [ntiles, P, 2 * FREE], _bf16, kind="ExternalInput")
    # host-precomputed one-hot rows, slot-major per tile: oh[t, p, j*K + k]
    # = 1.0 iff sample (t*SPT + p*SLOTS + j) has assignment k (pad rows are
    # all-zero).  Streaming these costs ~25 MB/core but removes every
    # per-slot DVE op from the kernel, leaving it DMA-bound.
    oh_d = nc.dram_tensor("oh", [ntiles, P, SLOTS * K], _bf16, kind="ExternalInput")
    s_out = nc.dram_tensor("s_out", [K, DA], _f32, kind="ExternalOutput")
    part_out = nc.dram_tensor("partials", [P, 2 * ntiles], _f32, kind="ExternalOutput")

    with ExitStack() as ctx:
        tc = ctx.enter_context(tile.TileContext(nc))
        const_pool = ctx.enter_context(tc.tile_pool(name="const", bufs=1))
        xin = ctx.enter_context(tc.tile_pool(name="xin", bufs=6))
        scratch = ctx.enter_context(tc.tile_pool(name="scratch", bufs=2))
        ohp = ctx.enter_context(tc.tile_pool(name="ohp", bufs=3))
        psum = ctx.enter_context(tc.tile_pool(name="psum", bufs=1, space="PSUM"))

        partials_sb = const_pool.tile([P, 2 * ntiles], _f32)

        s_psum = psum.tile([K, DA], _f32)

        for t in range(ntiles):
            xr_t = xin.tile([P, 2 * FREE], _bf16)
            nc.sync.dma_start(xr_t[:], xr_d[t, :, :])
            x_t = xr_t[:, 0:FREE]
            r_t = xr_t[:, FREE : 2 * FREE]

            d_t = scratch.tile([P, FREE], _bf16, tag="d")
            nc.vector.tensor_sub(d_t[:], r_t, x_t)
            sq_t = scratch.tile([P, FREE], _bf16, tag="sq")
            nc.scalar.activation(
                sq_t[:], d_t[:], mybir.ActivationFunctionType.Square,
                accum_out=partials_sb[:, t : t + 1],
            )
            sq2_t = scratch.tile([P, FREE], _bf16, tag="sq")
            nc.scalar.activation(
                sq2_t[:], x_t, mybir.ActivationFunctionType.Square,
                accum_out=partials_sb[:, ntiles + t : ntiles + t + 1],
            )

            oh_bf = ohp.tile([P, SLOTS * K], _bf16, tag="ohb")
            nc.sync.dma_start(oh_bf[:], oh_d[t, :, :])
            for j in range(SLOTS):
                nc.tensor.matmul(
                    s_psum[:],
                    oh_bf[:, j * K : (j + 1) * K],
                    x_t[:, j * DA : (j + 1) * DA],
                    start=(t == 0 and j == 0),
                    stop=(t == ntiles - 1 and j == SLOTS - 1),
                )

        s_sb = const_pool.tile([K, DA], _f32)
        nc.vector.tensor_copy(s_sb[:], s_psum[:])
        nc.sync.dma_start(s_out[:, :], s_sb[:])
        nc.sync.dma_start(part_out[:, :], partials_sb[:])

    nc.compile()
    return nc


def host_prepare(recon_x, x, cluster_assignments, ntiles: int = NTILES,
                 n_cores: int = N_CORES):
    """Shard + pad + cast + lay out the inputs for each core."""
    n_per_core = x.shape[0] // n_cores
    padded = ntiles * SPT
    x_np = np.asarray(x, dtype=np.float32).reshape(n_cores, n_per_core, D)
    r_np = np.asarray(recon_x, dtype=np.float32).reshape(n_cores, n_per_core, D)
    a_np = np.asarray(cluster_assignments).reshape(n_cores, n_per_core)

    xr = np.zeros((n_cores, ntiles, P, 2 * FREE), BF16)
    xa = np.zeros((n_cores, padded, DA), BF16)
    xa[:, :n_per_core, :D] = x_np.astype(BF16)
    xa[:, :n_per_core, D] = 1.0
    xr[:, :, :, 0:FREE] = xa.reshape(n_cores, ntiles, P, FREE)
    xa[:, :n_per_core, :D] = r_np.astype(BF16)   # reuse buffer for r-aug
    xr[:, :, :, FREE:] = xa.reshape(n_cores, ntiles, P, FREE)

    in_maps = []
    for c in range(n_cores):
        oh = np.zeros((padded, K), BF16)
        oh[np.arange(n_per_core), a_np[c].astype(np.int64)] = 1.0
        in_maps.append(
            {
                "xr": xr[c],
                "oh": oh.reshape(ntiles, P, SLOTS * K),
            }
        )
    return in_maps


def host_combine(results, cluster_centers, ntiles: int = NTILES,
                 n_real: int = N):
    """Reduce per-core outputs into (total, recon, cluster) in float64."""
    S = np.zeros((K, DA), np.float64)
    recon = 0.0
    xsq = 0.0
    for rd in results:
        S += rd["s_out"].astype(np.float64)
        pr = rd["partials"].astype(np.float64)
        recon += pr[:, :ntiles].sum()
        xsq += pr[:, ntiles:].sum()
    xsq -= n_real  # flag column contributes 1 per real sample
    cnt = S[:, D]
    C = np.asarray(cluster_centers, dtype=np.float64)
    cross = float((S[:, :D] * C).sum())
    w = (C * C).sum(axis=1)
    cluster = xsq - 2.0 * cross + float((cnt * w).sum())
    total = ALPHA * recon + BETA * cluster
    return (np.float32(total), np.float32(recon), np.float32(cluster))


_nc = None


def _get_nc():
    global _nc
    if _nc is None:
        _nc = build_nc()
    return _nc


def kernel(recon_x, x, cluster_assignments, cluster_centers):
    nc = _get_nc()
    in_maps = host_prepare(recon_x, x, cluster_assignments)
    res = run_bass_kernel_spmd(nc, in_maps, list(range(N_CORES)))
    return host_combine(res.results, cluster_centers)


# revision 24
# speedup vs baseline: 3.3783x; 1.0050x over previous
"""DeepClusterLoss on 8 Trainium2 NeuronCores (Bass/Tile).

reference:
    recon_loss   = sum((recon_x - x)**2)
    cluster_loss = sum((x - centers[assign])**2)
    total        = recon_loss + cluster_loss          (ALPHA = BETA = 1)

Device strategy (data-parallel over N, per the sharding hint):
  - Inputs are streamed in bf16 (host-side cast, exact-to-tolerance: all
    outputs are ~1e8-magnitude sums of ~1e0 terms; the bf16 rounding noise
    averages to ~1e-6 relative).  This halves HBM traffic and unlocks the
    fast PE/DVE paths (1 cycle/row matmuls, single-pass LDWEIGHTS, 2x DVE).
  - Each sample is stored as 65 bf16s: [x_i (64) | flag], flag = 1.0 for
    real samples, 0.0 for padding.  recon_x rows carry the same flag, so
    (r - x) has an exact 0 in the flag column.
  - recon part: DVE computes d = r - x (bf16), ACT computes Square(d) with
    accum_out -> fp32 per-partition partials.  ACT Square(x) likewise (the
    flag column adds +1 per real sample; the host subtracts N afterwards).
  - cluster part avoids the gather:
        cluster = sum|x|^2 - 2*sum_k <S_k, C_k> + sum_k n_k*|C_k|^2
    S_k (segment sums) and n_k (counts) come from ONE matmul per
    128-sample slot: a one-hot [128, K] bf16 (tensor_scalar is_equal
    against an iota row; built on DVE and GpSimd in parallel) contracted
    with the augmented x-slot [128, 65] -> PSUM [K, 65] fp32, where column
    64 (the flag) accumulates exactly n_k.
  - Host combines the tiny per-core fp32 outputs in float64.

Padding uses assignment class K (=100): its one-hot row is all zeros, so
padded samples vanish from S and the counts.
"""

import sys
from contextlib import ExitStack

import numpy as np

for _p in ("/opt/trn_rl_repo", "/opt/pypackages"):
    if _p not in sys.path:
        sys.path.append(_p)

import ml_dtypes
import concourse.tile as tile
from concourse import bacc, mybir
from concourse.bass_utils import run_bass_kernel_spmd

N, D, K = 1_000_000, 64, 100
ALPHA, BETA = 1.0, 1.0
N_CORES = 8
N_PER_CORE = N // N_CORES  # 125000
P = 128                    # SBUF partitions
DA = D + 1                 # augmented sample width (x | flag)
SLOTS = 32                 # sample-slots per partition per tile
FREE = DA * SLOTS          # 2080 bf16 per partition per half-tile
SPT = P * SLOTS            # samples per tile = 4096
NTILES = -(-N_PER_CORE // SPT)  # 31
PADDED = NTILES * SPT      # 126976
PAD_CLASS = float(K)       # out-of-range class: one-hot row is all zeros
GP_FRAC = 3                # j % GP_FRAC == 0 -> one-hot built on GpSimd

_bf16 = mybir.dt.bfloat16
_f32 = mybir.dt.float32
BF16 = ml_dtypes.bfloat16


def build_nc(ntiles: int = NTILES):
    """Build + compile the per-core Bass program (same program on all cores)."""
    nc = bacc.Bacc()
    # x-aug and r-aug interleaved per tile: xr[t, p, 0:FREE] = x-aug,
    # xr[t, p, FREE:2*FREE] = r-aug  (one DMA per tile)
    xr_d = nc.dram_tensor("xr", [ntiles, P, 2 * FREE], _bf16, kind="ExternalInput")
    # host-precomputed one-hot rows, slot-major per tile: oh[t, p, j*K + k]
    # = 1.0 iff sample (t*SPT + p*SLOTS + j) has assignment k (pad rows are
    # all-zero).  Streaming these costs ~25 MB/core but removes every
    # per-slot DVE op from the kernel, leaving it DMA-bound.
    oh_d = nc.dram_tensor("oh", [ntiles, P, SLOTS * K], _bf16, kind="ExternalInput")
    s_out = nc.dram_tensor("s_out", [K, DA], _f32, kind="ExternalOutput")
    part_out = nc.dram_tensor("partials", [P, 2 * ntiles], _f32, kind="ExternalOutput")

    with ExitStack() as ctx:
        tc = ctx.enter_context(tile.TileContext(nc))
        const_pool = ctx.enter_context(tc.tile_pool(name="const", bufs=1))
        xin = ctx.enter_context(tc.tile_pool(name="xin", bufs=6))
        scratch = ctx.enter_context(tc.tile_pool(name="scratch", bufs=2))
        ohp = ctx.enter_context(tc.tile_pool(name="ohp", bufs=5))
        psum = ctx.enter_context(tc.tile_pool(name="psum", bufs=1, space="PSUM"))

        partials_sb = const_pool.tile([P, 2 * ntiles], _f32)

        s_psum = psum.tile([K, DA], _f32)

        for t in range(ntiles):
            xr_t = xin.tile([P, 2 * FREE], _bf16)
            nc.sync.dma_start(xr_t[:], xr_d[t, :, :])
            x_t = xr_t[:, 0:FREE]
            r_t = xr_t[:, FREE : 2 * FREE]

            d_t = scratch.tile([P, FREE], _bf16, tag="d")
            nc.vector.tensor_sub(d_t[:], r_t, x_t)
            sq_t = scratch.tile([P, FREE], _bf16, tag="sq")
            nc.scalar.activation(
                sq_t[:], d_t[:], mybir.ActivationFunctionType.Square,
                accum_out=partials_sb[:, t : t + 1],
            )
            sq2_t = scratch.tile([P, FREE], _bf16, tag="sq")
            nc.scalar.activation(
                sq2_t[:], x_t, mybir.ActivationFunctionType.Square,
                accum_out=partials_sb[:, ntiles + t : ntiles + t + 1],
            )

            oh_bf = ohp.tile([P, SLOTS * K], _bf16, tag="ohb")
            # issue on the ACT HW-DGE ring so the xr stream (SP ring) and the
            # one-hot stream generate descriptors in parallel
            nc.scalar.dma_start(oh_bf[:], oh_d[t, :, :])
            for j in range(SLOTS):
                nc.tensor.matmul(
                    s_psum[:],
                    oh_bf[:, j * K : (j + 1) * K],
                    x_t[:, j * DA : (j + 1) * DA],
                    start=(t == 0 and j == 0),
                    stop=(t == ntiles - 1 and j == SLOTS - 1),
                )

        s_sb = const_pool.tile([K, DA], _f32)
        nc.vector.tensor_copy(s_sb[:], s_psum[:])
        nc.sync.dma_start(s_out[:, :], s_sb[:])
        nc.sync.dma_start(part_out[:, :], partials_sb[:])

    nc.compile()
    return nc


def host_prepare(recon_x, x, cluster_assignments, ntiles: int = NTILES,
                 n_cores: int = N_CORES):
    """Shard + pad + cast + lay out the inputs for each core."""
    n_per_core = x.shape[0] // n_cores
    padded = ntiles * SPT
    x_np = np.asarray(x, dtype=np.float32).reshape(n_cores, n_per_core, D)
    r_np = np.asarray(recon_x, dtype=np.float32).reshape(n_cores, n_per_core, D)
    a_np = np.asarray(cluster_assignments).reshape(n_cores, n_per_core)

    xr = np.zeros((n_cores, ntiles, P, 2 * FREE), BF16)
    xa = np.zeros((n_cores, padded, DA), BF16)
    xa[:, :n_per_core, :D] = x_np.astype(BF16)
    xa[:, :n_per_core, D] = 1.0
    xr[:, :, :, 0:FREE] = xa.reshape(n_cores, ntiles, P, FREE)
    xa[:, :n_per_core, :D] = r_np.astype(BF16)   # reuse buffer for r-aug
    xr[:, :, :, FREE:] = xa.reshape(n_cores, ntiles, P, FREE)

    in_maps = []
    for c in range(n_cores):
        oh = np.zeros((padded, K), BF16)
        oh[np.arange(n_per_core), a_np[c].astype(np.int64)] = 1.0
        in_maps.append(
            {
                "xr": xr[c],
                "oh": oh.reshape(ntiles, P, SLOTS * K),
            }
        )
    return in_maps


def host_combine(results, cluster_centers, ntiles: int = NTILES,
                 n_real: int = N):
    """Reduce per-core outputs into (total, recon, cluster) in float64."""
    S = np.zeros((K, DA), np.float64)
    recon = 0.0
    xsq = 0.0
    for rd in results:
        S += rd["s_out"].astype(np.float64)
        pr = rd["partials"].astype(np.float64)
        recon += pr[:, :ntiles].sum()
        xsq += pr[:, ntiles:].sum()
    xsq -= n_real  # flag column contributes 1 per real sample
    cnt = S[:, D]
    C = np.asarray(cluster_centers, dtype=np.float64)
    cross = float((S[:, :D] * C).sum())
    w = (C * C).sum(axis=1)
    cluster = xsq - 2.0 * cross + float((cnt * w).sum())
    total = ALPHA * recon + BETA * cluster
    return (np.float32(total), np.float32(recon), np.float32(cluster))


_nc = None


def _get_nc():
    global _nc
    if _nc is None:
        _nc = build_nc()
    return _nc


def kernel(recon_x, x, cluster_assignments, cluster_centers):
    nc = _get_nc()
    in_maps = host_prepare(recon_x, x, cluster_assignments)
    res = run_bass_kernel_spmd(nc, in_maps, list(range(N_CORES)))
    return host_combine(res.results, cluster_centers)
